# revision 1
# baseline (speedup 1.0000x reference)
"""MoD router Trainium2 kernel.

Computes, for hidden_states [4, 4096, 2048] and gate_w [1, 2048]:
    scores = einsum("bsh,h->bs", hidden_states, gate_w[0])        # [4, 4096]
    mask   = top-k mask per batch row (k = 2048 = S/2), 1.0/0.0   # [4, 4096]
returns (mask, scores), matching the reference.

Distribution: the B*S = 16384 score rows are sharded 8 ways (2048 rows per
NeuronCore, each core covering half of one batch row). Per core:
  1. DMA its [2048, 2048] f32 slab of hidden_states (16 MiB) in 2 MiB chunks,
     fused multiply+reduce against the (host-prebroadcast) gate vector on the
     vector engine -> 2048 scores.
  2. PE-transpose scores to flat row order, DMA out, AllGather all 16384
     scores across the 8 cores (8 KB/core payload).
  3. Select own batch row (one-hot matmul) and broadcast its 4096 scores to
     all 128 partitions; run a 4-level 127-ary threshold search: each level
     compares the row against 128 candidate pivots (fused compare+count in a
     single DVE op), then narrows [lo, hi] to the adjacent pivot pair that
     brackets count==k, bit-exactly (select-with-sentinel + max reduce).
  4. mask = (scores >= threshold), DMA out.
"""

import numpy as np

B, S, H = 4, 4096, 2048
N_CORES = 8
R = (B * S) // N_CORES      # rows per core = 2048
RT = R // 128               # 128-row tiles per core = 16
K_TOP = S // 2              # 2048
N_LEVELS = 3
LO0, HI0 = -8.0, 8.0

_CACHE = {}
_DEBUG = False
_PHASES = 6  # build phases 1..N (debug aid; 6 = full kernel)
_REPS = 1   # repeat whole body inside one NEFF (timing aid)


def _build_nc():
    import concourse.bacc as bacc
    import concourse.tile as tile
    import concourse.mybir as mybir

    f32 = mybir.dt.float32
    Alu = mybir.AluOpType
    Ax = mybir.AxisListType

    nc = bacc.Bacc("TRN2", target_bir_lowering=False, debug=False,
                   num_devices=N_CORES)

    h = nc.dram_tensor("h", [R, H], f32, kind="ExternalInput")
    wb = nc.dram_tensor("wb", [128, H], f32, kind="ExternalInput")
    sel = nc.dram_tensor("sel", [8, 256], f32, kind="ExternalInput")
    coef = nc.dram_tensor("coef", [2, 128], f32, kind="ExternalInput")
    # consts[:,0]=signs [1,-1]; consts[:,2]=-signs (cols 1,3 unused)
    consts = nc.dram_tensor("consts", [2, 4], f32, kind="ExternalInput")
    piv0 = nc.dram_tensor("piv0", [128, 2], f32, kind="ExternalInput")
    ident = nc.dram_tensor("ident", [128, 128], f32, kind="ExternalInput")
    scores_out = nc.dram_tensor("scores_out", [RT, 128], f32,
                                kind="ExternalOutput")
    mask_out = nc.dram_tensor("mask_out", [RT, 128], f32,
                              kind="ExternalOutput")
    if _DEBUG:
        dbg_bcast = nc.dram_tensor("dbg_bcast", [128, 4096], f32,
                                   kind="ExternalOutput")
        dbg_lvl = nc.dram_tensor("dbg_lvl", [N_LEVELS, 2, 2], f32,
                                 kind="ExternalOutput")
        dbg_cnt = nc.dram_tensor("dbg_cnt", [N_LEVELS, 128, 2], f32,
                                 kind="ExternalOutput")

    with tile.TileContext(nc) as tc:
        with (
            tc.tile_pool(name="big", bufs=6) as big,
            tc.tile_pool(name="junk", bufs=2) as junkp,
            tc.tile_pool(name="small", bufs=1) as small,
            tc.tile_pool(name="psum", bufs=1, space="PSUM") as psum,
            tc.tile_pool(name="dram", bufs=1, space="DRAM") as dram,
        ):
            w_sb = small.tile([128, H], f32)
            nc.sync.dma_start(w_sb[:], wb.ap())
            sel_sb = small.tile([8, 256], f32)
            nc.sync.dma_start(sel_sb[:], sel.ap())
            coef_sb = small.tile([2, 128], f32)
            nc.sync.dma_start(coef_sb[:], coef.ap())
            consts_sb = small.tile([2, 4], f32)
            nc.sync.dma_start(consts_sb[:], consts.ap())
            piv_sb = small.tile([128, 2], f32)
            nc.sync.dma_start(piv_sb[:], piv0.ap())
            id_sb = small.tile([128, 128], f32)
            nc.sync.dma_start(id_sb[:], ident.ap())

            for rep in range(_REPS):
                scores_sb = small.tile([128, RT], f32)

                # ---- phase 1: matvec ----
                NA = 1  # row-tiles per DMA chunk (chunk = NA MiB)
                hv = h.ap().rearrange("(n a p) d -> n p a d", a=NA, p=128)
                for i in range(RT // NA):
                    ht = big.tile([128, NA, H], f32, tag="ht")
                    nc.sync.dma_start(ht[:], hv[i])
                    for a in range(NA):
                        junk = junkp.tile([128, H], f32, tag="junk")
                        nc.vector.scalar_tensor_tensor(
                            junk[:], ht[:, a, :], 0.0, w_sb[:],
                            op0=Alu.bypass, op1=Alu.mult,
                            accum_out=scores_sb[:, i * NA + a: i * NA + a + 1],
                        )

                # ---- phase 2: scores -> flat row order ----
                ps_t = psum.tile([RT, 128], f32, tag="ps")
                nc.tensor.transpose(ps_t[:], scores_sb[:], id_sb[:])
                flat_sc = small.tile([RT, 128], f32)
                nc.vector.tensor_copy(flat_sc[:], ps_t[:])
                nc.sync.dma_start(scores_out.ap(), flat_sc[:])

                mask_written = False

                # ---- phase 3: AllGather scores ----
                if _PHASES >= 3:
                    ag_in = dram.tile([RT, 128], f32)
                    ag_out = dram.tile([8, R], f32)
                    nc.sync.dma_start(ag_in[:], flat_sc[:])
                    nc.gpsimd.collective_compute(
                        "AllGather", Alu.bypass,
                        replica_groups=[list(range(N_CORES))],
                        ins=[ag_in.opt()], outs=[ag_out.opt()],
                    )
                    ag_sb = small.tile([8, R], f32)
                    nc.sync.dma_start(ag_sb[:], ag_out[:])

                # ---- phase 4: select own batch row, broadcast to partitions ----
                if _PHASES >= 4:
                    ps_b = psum.tile([128, 4096], f32, tag="ps")
                    for j in range(8):
                        hh, nn = j // 4, j % 4
                        nc.tensor.matmul(
                            ps_b[:, j * 512:(j + 1) * 512],
                            sel_sb[:, hh * 128:(hh + 1) * 128],
                            ag_sb[:, nn * 512:(nn + 1) * 512],
                        )
                    bcast = small.tile([128, 4096], f32)
                    nc.vector.tensor_copy(bcast[:], ps_b[:])
                    if _DEBUG:
                        nc.sync.dma_start(dbg_bcast.ap(), bcast[:])

                # ---- phase 5: 127-ary threshold search ----
                if _PHASES >= 5:
                    cnt = small.tile([128, 1], f32)
                    cond = small.tile([128, 1], mybir.dt.int32)
                    ncond = small.tile([128, 1], mybir.dt.int32)
                    mm = small.tile([128, 2], f32)
                    negbig = small.tile([128, 2], f32)
                    nc.vector.memset(negbig[:], -1.0e30)
                    lohi_raw = small.tile([2, 1], f32)
                    lohi2 = small.tile([2, 2], f32)
                    for lvl in range(N_LEVELS):
                        junkb = big.tile([128, 4096], f32, tag="ht")
                        nc.vector.tensor_scalar(
                            junkb[:], bcast[:], piv_sb[:, 0:1], None,
                            op0=Alu.is_ge, op1=Alu.add, accum_out=cnt[:],
                        )
                        nc.vector.tensor_scalar(cond[:], cnt[:], float(K_TOP),
                                                None, op0=Alu.is_ge)
                        nc.vector.tensor_scalar(ncond[:], cnt[:], float(K_TOP),
                                                None, op0=Alu.is_lt)
                        # Bit-exact select: mm[:,0] = cond ? piv : -BIG
                        #                   mm[:,1] = ncond ? -piv : -BIG
                        # so max(mm[:,0]) = lo', max(mm[:,1]) = -hi'.
                        nc.vector.tensor_copy(mm[:], negbig[:])
                        nc.vector.copy_predicated(mm[:, 0:1], cond[:],
                                                  piv_sb[:, 0:1])
                        nc.vector.copy_predicated(mm[:, 1:2], ncond[:],
                                                  piv_sb[:, 1:2])
                        ps_m = psum.tile([2, 128], f32, tag="ps")
                        nc.tensor.transpose(ps_m[:], mm[:], id_sb[:])
                        nc.vector.tensor_reduce(lohi_raw[:], ps_m[:], axis=Ax.X,
                                                op=Alu.max)
                        # lohi2[:,0] = raw*sign = [lo', hi']  (signs [1,-1])
                        # lohi2[:,1] = -lohi2[:,0]
                        nc.vector.tensor_scalar(
                            lohi2[:, 0:1], lohi_raw[:], consts_sb[:, 0:1], None,
                            op0=Alu.mult)
                        nc.vector.tensor_scalar(
                            lohi2[:, 1:2], lohi_raw[:], consts_sb[:, 2:3], None,
                            op0=Alu.mult)
                        if _DEBUG:
                            nc.sync.dma_start(dbg_lvl.ap()[lvl], lohi2[:])
                            dc = small.tile([128, 2], f32, tag=f"dc{lvl}")
                            nc.vector.tensor_copy(dc[:, 0:1], cnt[:])
                            nc.vector.tensor_copy(dc[:, 1:2], piv_sb[:, 0:1])
                            nc.sync.dma_start(dbg_cnt.ap()[lvl], dc[:])
                        if lvl < N_LEVELS - 1:
                            ps_p = psum.tile([128, 2], f32, tag="ps")
                            nc.tensor.matmul(ps_p[:], coef_sb[:], lohi2[:])
                            nc.vector.tensor_copy(piv_sb[:], ps_p[:])

                # ---- phase 6: mask = scores >= tau ----
                if _PHASES >= 6:
                    ones16 = small.tile([1, RT], f32)
                    nc.vector.memset(ones16[:], 1.0)
                    ps_tau = psum.tile([RT, 1], f32, tag="ps")
                    nc.tensor.matmul(ps_tau[:], ones16[:], lohi2[0:1, 0:1])
                    mask_sb = small.tile([RT, 128], f32)
                    nc.vector.tensor_scalar(mask_sb[:], flat_sc[:],
                                            ps_tau[:], None, op0=Alu.is_ge)
                    nc.sync.dma_start(mask_out.ap(), mask_sb[:])
                    mask_written = True

                if not mask_written:
                    mask_sb = small.tile([RT, 128], f32)
                    nc.vector.memset(mask_sb[:], 0.0)
                    nc.sync.dma_start(mask_out.ap(), mask_sb[:])

    nc.compile()
    return nc


def _host_inputs(hidden_states, gate_w):
    flat = np.ascontiguousarray(
        np.asarray(hidden_states, dtype=np.float32).reshape(B * S, H))
    wb = np.ascontiguousarray(
        np.broadcast_to(np.asarray(gate_w, dtype=np.float32).reshape(1, H),
                        (128, H)))
    coef = np.empty((2, 128), np.float32)
    p = np.arange(128, dtype=np.float32)
    coef[1] = p / np.float32(127.0)
    coef[0] = np.float32(1.0) - coef[1]
    consts = np.array([[1.0, -10.0, -1.0, 10.0],
                       [-1.0, 10.0, 1.0, -10.0]], np.float32)
    piv0 = np.empty((128, 2), np.float32)
    piv0[:, 0] = np.float32(LO0) + p * np.float32((HI0 - LO0) / 127.0)
    piv0[:, 1] = -piv0[:, 0]
    ident = np.eye(128, dtype=np.float32)

    in_maps = []
    for c in range(N_CORES):
        b = c // 2
        sel = np.zeros((8, 256), np.float32)
        sel[2 * b, :128] = 1.0
        sel[2 * b + 1, 128:] = 1.0
        in_maps.append({
            "h": flat[c * R:(c + 1) * R],
            "wb": wb,
            "sel": sel,
            "coef": coef,
            "consts": consts,
            "piv0": piv0,
            "ident": ident,
        })
    return in_maps


def _assemble(results):
    scores = np.concatenate(
        [results[c]["scores_out"].reshape(R) for c in range(N_CORES)]
    ).reshape(B, S)
    mask = np.concatenate(
        [results[c]["mask_out"].reshape(R) for c in range(N_CORES)]
    ).reshape(B, S)
    return mask, scores


def get_nc():
    if "nc" not in _CACHE:
        _CACHE["nc"] = _build_nc()
    return _CACHE["nc"]


def kernel(hidden_states, gate_w):
    from concourse.bass_utils import run_bass_kernel_spmd

    nc = get_nc()
    in_maps = _host_inputs(hidden_states, gate_w)
    res = run_bass_kernel_spmd(nc, in_maps, core_ids=list(range(N_CORES)))
    return _assemble(res.results)

